# revision 38
# baseline (speedup 1.0000x reference)
"""MemoryBank kernel for 8x TRN2 NeuronCores (SPMD, batch-parallel).

Algebraic restructure (exact in real arithmetic):
    S        = x @ [Ws; gx_w].T          # [L, 65]; Ws = memory @ key_w (folded)
    E        = exp(10*(S[:, :64] - m))   # m = per-token max
    sum      = E.1,  r = 1/sum
    gl       = (E @ gv) * r + S[:, 64] + b      # gv = Wv @ gate_w[D:]
    u        = exp(-gl); g = 1/(1+u)            # sigmoid without Sigmoid LUT
    out      = g * (x + (E * u * r) @ Wv)       # since (1-g)/g == u exactly

Numerics: x is split on the host into xh (fp16) + xl (residual). Scores are
computed at fp32-grade precision as three PE passes, all scaled by 2^12 so
the corrections fit fp8 range (the scale is unwound in the exp/gate stats):
  (2^12*Gh)@xh [fp16, full rate] + fp8(2^12*Gl)@fp8(xh) + fp8(Gh)@fp8(2^12*xl)
with both correction passes in fp8 DoubleRow mode (2 chunks per matmul at 2x
rate). fp8(xh) is cast on-device on the Scalar engine to save HBM traffic.

Combine: the PE accumulates xh (identity matmul) and the retrieval (fp16,
K padded to 128 with zero rows against a once-zeroed E buffer) into the same
PSUM bank, so the elementwise work collapses to one DVE multiply by a
broadcast gate row per bank pair; output is written fp16 (host upcasts).

Schedule per iteration t (software pipelined, all DMAs fully contiguous):
  PE : id-quad0(t) -> ET(t) -> g_bc(t) -> [P quads(t) + S(t+1) interleaved,
       same-bank matmuls >= 6 apart] -> Stok(t+1) transposes
  DVE: combine muls(t) -> stats(t+1)  (emitted after quad-2 TTs)
  ACT: E_copy(t), g2(t), S_copy(t+1), exps(t+1), fp8-cast(t+2)
  GpS: output DMA issue (SWDGE)      Sync: input DMA issue (HWDGE)
Each core handles one batch element; x is pre-tiled on the host so every
transfer is a single 1-2 MiB contiguous DMA.
"""

from contextlib import ExitStack

import numpy as np

F16_NP = np.float16
import ml_dtypes

F8_NP = ml_dtypes.float8_e4m3

import concourse.bass as bass
import concourse.tile as tile
from concourse import bacc
from concourse import mybir
from concourse.bass import ts
from concourse.bass_utils import run_bass_kernel_spmd
from concourse.masks import make_identity

F32 = mybir.dt.float32
F32R = mybir.dt.float32r
F16 = mybir.dt.float16
F8 = mybir.dt.float8e4
DR = mybir.MatmulPerfMode.DoubleRow
SIG = 4096.0  # power-of-2 score scale so the fp8 correction passes fit fp8 range
AX_X = mybir.AxisListType
ALU = mybir.AluOpType
ACTF = mybir.ActivationFunctionType

B = 8
L = 4096
DIM = 2048
NSLOT = 64
NCH = DIM // 128  # 16 dim chunks
TOK = 512  # tokens per tile
NT = L // TOK  # 8 tiles per core
NQ = TOK // 128  # 4 token quarters per tile


def _build(gate_b: float, interleave: bool = True) -> bass.Bass:
    nc = bacc.Bacc("TRN2", target_bir_lowering=False, debug=False)

    xhd = nc.dram_tensor("xhd", [NT * 128, NCH * TOK], F16, kind="ExternalInput").ap()
    xl8d = nc.dram_tensor(
        "xl8d", [NT * 128, NCH * TOK], F8, kind="ExternalInput"
    ).ap()
    Ghd = nc.dram_tensor(
        "Ghd", [128, NCH * 128], F16, kind="ExternalInput"
    ).ap()
    Gh8d = nc.dram_tensor("Gh8d", [128, NCH * 128], F8, kind="ExternalInput").ap()
    Gl8d = nc.dram_tensor("Gl8d", [128, NCH * 128], F8, kind="ExternalInput").ap()
    Wvd = nc.dram_tensor("Wvd", [128, NCH * 128], F16, kind="ExternalInput").ap()
    gvd = nc.dram_tensor("gvd", [1, NQ * NSLOT], F32, kind="ExternalInput").ap()
    outb = nc.dram_tensor(
        "outb", [NT * 128, NCH * TOK], F16, kind="ExternalOutput"
    ).ap()

    # per-tile views; dim d = c*128 + p everywhere
    xh_v = xhd.rearrange("(t p) (c k) -> t p c k", p=128, k=TOK)
    xl8_v = xl8d.rearrange("(t p) (c k) -> t p c k", p=128, k=TOK)
    outb_v = outb.rearrange("(t p) (c k) -> t p c k", p=128, k=TOK)
    Gh_v = Ghd.rearrange("p (c n) -> p c n", n=128)
    Gh8_v = Gh8d.rearrange("p (c n) -> p c n", n=128)
    Gl8_v = Gl8d.rearrange("p (c n) -> p c n", n=128)
    Wv_v = Wvd.rearrange("n (c j) -> n c j", j=128)

    with tile.TileContext(nc) as tc, ExitStack() as ctx:
        consts = ctx.enter_context(tc.tile_pool(name="consts", bufs=1))
        xpool = ctx.enter_context(tc.tile_pool(name="xpool", bufs=4))
        x8pool = ctx.enter_context(tc.tile_pool(name="x8pool", bufs=3))
        opool = ctx.enter_context(tc.tile_pool(name="opool", bufs=2))
        work = ctx.enter_context(tc.tile_pool(name="work", bufs=2))
        small = ctx.enter_context(tc.tile_pool(name="small", bufs=2))
        psS = ctx.enter_context(tc.tile_pool(name="psS", bufs=1, space="PSUM"))
        psT = ctx.enter_context(tc.tile_pool(name="psT", bufs=1, space="PSUM"))
        psE = ctx.enter_context(tc.tile_pool(name="psE", bufs=1, space="PSUM"))
        psG = ctx.enter_context(tc.tile_pool(name="psG", bufs=1, space="PSUM"))
        psP = ctx.enter_context(tc.tile_pool(name="psP", bufs=2, space="PSUM"))

        ident = consts.tile([128, 128], F32)
        make_identity(nc, ident)
        ident_16 = consts.tile([128, 128], F16)
        nc.vector.tensor_copy(ident_16, ident)
        Gh_sb = consts.tile([128, NCH, 128], F16)
        Gh8_sb = consts.tile([128, NCH, 128], F8)
        Gl8_sb = consts.tile([128, NCH, 128], F8)
        Wv_sb = consts.tile([128, NCH, 128], F16)
        # manually double-buffered full-height E tiles; rows 65:128 are
        # zeroed once and never rewritten, so the zero-padded Wv rows
        # always multiply zeros (K=128 keeps the weight load on the
        # fast path)
        E_pair = []
        for i in range(2):
            E_buf = consts.tile([128, NQ, 128], F16, name=f"E_buf{i}")
            nc.vector.memset(E_buf, 0.0)
            E_pair.append(E_buf)
        gv4_flat = consts.tile([128, NQ * NSLOT], F32)
        gv4 = gv4_flat.rearrange("p (a b) -> p a b", b=NSLOT)
        ones_16 = consts.tile([NSLOT + 1, 128], F16)
        nc.vector.memset(ones_16, 1.0)

        def ph_dma(t):
            xh_sb = xpool.tile([128, NCH, TOK], F16, tag="xh")
            nc.sync.dma_start(out=xh_sb, in_=xh_v[t])
            xl8_sb = x8pool.tile([128, NCH, TOK], F8, tag="xl8")
            nc.sync.dma_start(out=xl8_sb, in_=xl8_v[t])
            return {"xh": xh_sb, "xl8": xl8_sb}

        def ph_cast(t, st):
            # fp8 copy of xh on ACT (saves 1 MiB/tile of HBM traffic)
            xh8_sb = x8pool.tile([128, NCH, TOK], F8, tag="xh8")
            nc.scalar.copy(xh8_sb, st["xh"])
            st["xh8"] = xh8_sb

        def s_seq(st):
            # sigma-scaled split: SIG*G@x = (SIG*Gh)@xh [f16]
            #   + fp8(SIG*Gl)@fp8(xh) + fp8(Gh)@fp8(SIG*xl)  [fp8 DoubleRow,
            #   2 chunks per matmul at 2x rate]
            seq = []
            for c in range(NCH):
                seq.append((Gh_sb[:, c, :], st["xh"][:, c, :], None))
            for G8, xkey in ((Gl8_sb, "xh8"), (Gh8_sb, "xl8")):
                for c2 in range(NCH // 2):
                    seq.append(
                        (
                            G8[:, 2 * c2 : 2 * c2 + 2, :],
                            st[xkey][:, 2 * c2 : 2 * c2 + 2, :],
                            DR,
                        )
                    )
            return seq

        def ph_S(t, st):
            """S^T = [Ws; gxw] @ x_tile via 3 fp16 passes per chunk."""
            S_ps = psS.tile([128, TOK], F32, tag="S")
            seq = s_seq(st)
            for i, (lhsT, rhs, pm) in enumerate(seq):
                nc.tensor.matmul(
                    S_ps,
                    lhsT,
                    rhs,
                    start=(i == 0),
                    stop=(i == len(seq) - 1),
                    perf_mode=pm,
                )
            st["S_ps"] = S_ps

        def ph_Scopy(t, st):
            S_ps = st.pop("S_ps")
            S_sb = work.tile([NSLOT + 1, TOK], F32, tag="S_sb")
            nc.scalar.copy(S_sb, S_ps[0 : NSLOT + 1, :])
            st["S_sb"] = S_sb

        def ph_Stok_T(t, st):
            S_sb = st.pop("S_sb")
            Stok = psT.tile([128, NQ, NSLOT + 1], F32, tag="T")
            for q in range(NQ):
                nc.tensor.transpose(
                    Stok[:, q, :],
                    S_sb[:, ts(q, 128)],
                    ident[0 : NSLOT + 1, 0 : NSLOT + 1],
                )
            st["Stok"] = Stok

        def ph_Stok(t, st):
            ph_Scopy(t, st)
            ph_Stok_T(t, st)

        def ph_stats(t, st):
            """Token-major softmax stats; produces Ec (bf16, col 64 = g)."""
            Stok = st.pop("Stok")
            mx4 = small.tile([128, NQ], F32, tag="mx4")
            nc.vector.tensor_reduce(mx4, Stok[:, :, 0:NSLOT], axis=AX_X.X, op=ALU.max)
            mb4 = small.tile([128, NQ], F32, tag="mb4")
            nc.vector.tensor_scalar_mul(mb4, mx4, -10.0 / SIG)
            Etok = work.tile([128, NQ, NSLOT], F32, tag="Etok")
            s4 = small.tile([128, NQ], F32, tag="s4")
            for q in range(NQ):
                nc.scalar.activation(
                    Etok[:, q, :],
                    Stok[:, q, 0:NSLOT],
                    func=ACTF.Exp,
                    bias=mb4[:, q : q + 1],
                    scale=10.0 / SIG,
                    accum_out=s4[:, q : q + 1],
                )
            gvd4 = small.tile([128, NQ], F32, tag="gvd4")
            scr = work.tile([128, NQ, NSLOT], F32, tag="scr")
            nc.vector.tensor_mul(scr, Etok, gv4)
            nc.vector.tensor_reduce(gvd4, scr, axis=AX_X.X, op=ALU.add)
            r4 = small.tile([128, NQ], F32, tag="r4")
            nc.vector.reciprocal(r4, s4)
            t4 = small.tile([128, NQ], F32, tag="t4")
            nc.vector.tensor_mul(t4, gvd4, r4)
            gxs = small.tile([128, NQ], F32, tag="gxs")
            gx_row = Stok[:, :, NSLOT : NSLOT + 1].rearrange("p a b -> p (a b)")
            nc.vector.tensor_scalar_mul(gxs, gx_row, 1.0 / SIG)
            gl4 = small.tile([128, NQ], F32, tag="gl4")
            nc.vector.tensor_add(gl4, t4, gxs)
            u4 = small.tile([128, NQ], F32, tag="u4")
            nc.scalar.activation(
                u4, gl4, func=ACTF.Exp, bias=-gate_b, scale=-1.0
            )
            den4 = small.tile([128, NQ], F32, tag="den4")
            nc.vector.tensor_scalar_add(den4, u4, 1.0)
            g4 = small.tile([128, NQ], F32, tag="g4")
            nc.vector.reciprocal(g4, den4)
            cp4 = small.tile([128, NQ], F32, tag="cp4")
            nc.vector.tensor_mul(cp4, u4, r4)
            Ec = work.tile([128, NQ, NSLOT + 1], F16, tag="Ec")
            for q in range(NQ):
                nc.vector.tensor_scalar_mul(
                    Ec[:, q, 0:NSLOT], Etok[:, q, :], cp4[:, q : q + 1]
                )
            g_col = Ec[:, :, NSLOT : NSLOT + 1].rearrange("p a b -> p (a b)")
            nc.vector.tensor_copy(g_col, g4)
            st["Ec"] = Ec

        def ph_combine(t, st, s_next=None, stats_cb=None):
            """P = x + E''@Wv on PE in quads (2 PSUM pairs in flight ->
            same-bank matmuls >=6 apart); out = g * P on DVE. The
            iteration's PE stream opens with dependency-free ident
            matmuls while DVE finishes the previous stats; the next
            tile's S matmuls fill quads 0-2 and its stats are emitted
            mid-iteration."""
            x = st["xh"]
            Ec = st.pop("Ec")
            E_sb = E_pair[t % 2]
            E_flat = E_sb.rearrange("p a b -> p (a b)")  # [128, 512]
            if s_next is not None:
                t2, st2 = s_next
                st2["_Sps"] = psS.tile([128, TOK], F32, tag="S", name="S_ps_il")
                st2["_seq"] = s_seq(st2)

            def s_il(n):
                if s_next is None:
                    return
                seq = st2["_seq"]
                lo = st2.setdefault("_si", 0)
                hi = min(lo + n, len(seq))
                st2["_si"] = hi
                for i2 in range(lo, hi):
                    lhsT, rhs, pm = seq[i2]
                    nc.tensor.matmul(
                        st2["_Sps"],
                        lhsT,
                        rhs,
                        start=(i2 == 0),
                        stop=(i2 == len(seq) - 1),
                        perf_mode=pm,
                        skip_group_check=True,
                    )
                if lo < len(seq) <= hi:
                    st2["S_ps"] = st2.pop("_Sps")
                    ph_Scopy(t2, st2)

            out_sb = opool.tile([128, NCH, TOK], F16, tag="o")
            quads = []
            for qd in range(4):
                Pa = psP.tile([128, 2, TOK], F32, tag="P", name="Pa")
                Pb = psP.tile([128, 2, TOK], F32, tag="P", name="Pb")
                quads.append((Pa, Pb))
                c0 = 4 * qd
                for k in range(2):
                    nc.tensor.matmul(
                        Pa[:, k, :], ident_16, x[:, c0 + k, :],
                        start=True, stop=False,
                    )
                for k in range(2):
                    nc.tensor.matmul(
                        Pb[:, k, :], ident_16, x[:, c0 + 2 + k, :],
                        start=True, stop=False,
                    )
                if qd == 0:
                    # ET transposes + g broadcast ride here: the idents
                    # above gave DVE time to finish this tile's stats
                    ET = psE.tile([128, NQ, 128], F16, tag="ET")
                    for q in range(NQ):
                        nc.tensor.transpose(
                            ET[0 : NSLOT + 1, q, :], Ec[:, q, :], ident_16
                        )
                    nc.scalar.copy(
                        E_sb[0 : NSLOT + 1, :, :], ET[0 : NSLOT + 1, :, :]
                    )
                    g_ps = psG.tile([128, TOK], F32, tag="G")
                    nc.tensor.matmul(
                        g_ps,
                        ones_16[NSLOT : NSLOT + 1, :],
                        E_flat[NSLOT : NSLOT + 1, :],
                        start=True,
                        stop=True,
                    )
                    g2 = work.tile([128, 2, TOK], F32, tag="g2")
                    nc.scalar.copy(g2[:, 0, :], g_ps)
                    nc.scalar.copy(g2[:, 1, :], g_ps)
                else:
                    s_il(11)
                for k in range(2):
                    nc.tensor.matmul(
                        Pa[:, k, :], Wv_sb[:, c0 + k, :], E_flat,
                        start=False, stop=True,
                    )
                for k in range(2):
                    nc.tensor.matmul(
                        Pb[:, k, :], Wv_sb[:, c0 + 2 + k, :], E_flat,
                        start=False, stop=True,
                    )
                s_il(10 if qd == 0 else 11)
                if qd == 2 and s_next is not None:
                    ph_Stok_T(t2, st2)
                nc.vector.tensor_mul(out_sb[:, c0 : c0 + 2, :], Pa, g2)
                nc.vector.tensor_mul(out_sb[:, c0 + 2 : c0 + 4, :], Pb, g2)
                if qd == 1:
                    nc.gpsimd.dma_start(
                        out=outb_v[t][:, 0:8, :], in_=out_sb[:, 0:8, :]
                    )
                elif qd == 2:
                    if stats_cb is not None:
                        stats_cb()
                    if s_next is None:
                        nc.gpsimd.dma_start(
                            out=outb_v[t][:, 8:12, :], in_=out_sb[:, 8:12, :]
                        )
            if s_next is None:
                nc.gpsimd.dma_start(
                    out=outb_v[t][:, 12:16, :], in_=out_sb[:, 12:16, :]
                )
            else:
                nc.gpsimd.dma_start(
                    out=outb_v[t][:, 8:16, :], in_=out_sb[:, 8:16, :]
                )
            if s_next is not None:
                st2.pop("_seq")
                st2.pop("_si", None)

        # Software pipeline: per iteration t the PE stream is
        #   ET(t) -> g_bc(t) -> [P pairs(t) + S(t+1) interleaved] -> Stok(t+1)
        # DVE: combine TTs(t) -> stats(t+1);  x DMA runs 2 tiles ahead.
        # Prologue DMAs are issued in latency-critical order so S(0) can
        # start after just xh(0) + Gh (~2.5 MiB).
        st0 = {}
        st0["xh"] = xpool.tile([128, NCH, TOK], F16, tag="xh", name="xh0")
        nc.sync.dma_start(out=st0["xh"], in_=xh_v[0])
        nc.sync.dma_start(out=Gh_sb, in_=Gh_v)
        st0["xl8"] = x8pool.tile([128, NCH, TOK], F8, tag="xl8", name="xl80")
        nc.sync.dma_start(out=st0["xl8"], in_=xl8_v[0])
        nc.sync.dma_start(out=Gh8_sb, in_=Gh8_v)
        nc.sync.dma_start(out=Gl8_sb, in_=Gl8_v)
        nc.sync.dma_start(out=Wv_sb, in_=Wv_v)
        nc.sync.dma_start(out=gv4_flat, in_=gvd.to_broadcast((128, NQ * NSLOT)))
        # dependency-free warm-up matmuls while the first DMAs land:
        # ~5us of sustained PE activity releases the HAM clock gate
        # (1.2 -> 2.4 GHz) before S(0) issues
        for w in range(72):
            g_warm = psG.tile([128, 128], F32, tag="G", name="g_warm")
            nc.tensor.matmul(g_warm, ident_16, ident_16, start=True, stop=True)
        states = {0: st0, 1: ph_dma(1), 2: ph_dma(2)}
        ph_cast(0, states[0])
        ph_cast(1, states[1])
        ph_S(0, states[0])
        ph_cast(2, states[2])
        ph_Stok(0, states[0])
        ph_stats(0, states[0])
        for t in range(NT):
            if t + 3 < NT:
                states[t + 3] = ph_dma(t + 3)
            cb = None
            if t + 1 < NT and interleave:
                cb = lambda t1=t + 1: ph_stats(t1, states[t1])
            ph_combine(
                t,
                states[t],
                s_next=(t + 1, states[t + 1])
                if (interleave and t + 1 < NT)
                else None,
                stats_cb=cb,
            )
            if 0 < t and t + 2 < NT:
                ph_cast(t + 2, states[t + 2])
            if t + 1 < NT and not interleave:
                ph_S(t + 1, states[t + 1])
                ph_Stok(t + 1, states[t + 1])
                ph_stats(t + 1, states[t + 1])
            del states[t]

    nc.compile()
    return nc


def _fold_weights(memory, key_w, value_w, gate_w):
    mem = np.asarray(memory, np.float64)
    # query = x @ key_w.T ; scores = query @ memory.T = x @ (memory @ key_w).T
    Ws = (mem @ np.asarray(key_w, np.float64)).astype(np.float32)  # [64, 2048]
    Wv = (mem @ np.asarray(value_w, np.float64).T).astype(np.float32)  # [64, 2048]
    gx = np.asarray(gate_w[0, :DIM], dtype=np.float32)
    gv = (Wv.astype(np.float64) @ np.asarray(gate_w[0, DIM:], np.float64)).astype(
        np.float32
    )
    G = np.concatenate(
        [Ws, gx[None, :], np.zeros((128 - NSLOT - 1, DIM), np.float32)], axis=0
    )  # [128, 2048]: 64 slots, gate row, zero padding (FWL wants 128 cols)
    GT = np.ascontiguousarray(
        G.T.reshape(NCH, 128, 128).transpose(1, 0, 2)
    ).reshape(128, NCH * 128)
    Gh = GT.astype(np.float16)
    GhF = Gh.astype(np.float32)
    Ghs = (GhF * 4096.0).astype(np.float16)  # exact power-of-2 shift
    Gh8 = GhF.astype(F8_NP)
    Gl8 = ((GT - GhF) * 4096.0).astype(F8_NP)
    WvT = np.ascontiguousarray(Wv.reshape(NSLOT, NCH * 128))  # [64, 2048]
    gv4 = np.tile(gv, NQ).reshape(1, NQ * NSLOT)
    return Ghs, Gh8, Gl8, WvT, gv4


def _tile_x(xb):
    # [L, D] -> [NT*128, NCH*TOK]: blob[t, p, c, k] = x[t*TOK+k, c*128+p]
    return np.ascontiguousarray(
        xb.reshape(NT, TOK, NCH, 128).transpose(0, 3, 2, 1)
    ).reshape(NT * 128, NCH * TOK)


def _untile_out(blob):
    # [NT*128, NCH*TOK] -> [L, D]
    return (
        blob.reshape(NT, 128, NCH, TOK)
        .transpose(0, 3, 2, 1)
        .reshape(L, DIM)
        .astype(np.float32)
    )


def kernel(
    x,
    memory,
    key_w,
    value_w,
    gate_w,
    gate_b,
    _trace=False,
    _tmpdir=None,
):
    x = np.asarray(x, dtype=np.float32)
    Ghs, Gh8, Gl8, WvT, gv4 = _fold_weights(
        np.asarray(memory, np.float32),
        np.asarray(key_w, np.float32),
        np.asarray(value_w, np.float32),
        np.asarray(gate_w, np.float32),
    )
    Wv_16 = np.concatenate([WvT, np.zeros_like(WvT)], axis=0).astype(F16_NP)
    nc = _build(float(np.asarray(gate_b).reshape(-1)[0]))
    in_maps = []
    for b in range(B):
        xt = _tile_x(x[b])
        xh = xt.astype(np.float16)
        xl8 = ((xt - xh.astype(np.float32)) * 4096.0).astype(F8_NP)
        in_maps.append(
            {
                "xhd": xh,
                "xl8d": xl8,
                "Ghd": Ghs,
                "Gh8d": Gh8,
                "Gl8d": Gl8,
                "Wvd": Wv_16,
                "gvd": gv4,
            }
        )
    res = run_bass_kernel_spmd(
        nc, in_maps, list(range(B)), trace=_trace, tmpdir=_tmpdir
    )
    out = np.stack(
        [_untile_out(np.asarray(res.results[b]["outb"])) for b in range(B)], axis=0
    )
    if _trace:
        return out.astype(np.float32), res
    return out.astype(np.float32)


# revision 40
# speedup vs baseline: 1.0369x; 1.0369x over previous
"""MemoryBank kernel for 8x TRN2 NeuronCores (SPMD, batch-parallel).

Algebraic restructure (exact in real arithmetic):
    S        = x @ [Ws; gx_w].T          # [L, 65]; Ws = memory @ key_w (folded)
    E        = exp(10*(S[:, :64] - m))   # m = per-token max
    sum      = E.1,  r = 1/sum
    gl       = (E @ gv) * r + S[:, 64] + b      # gv = Wv @ gate_w[D:]
    u        = exp(-gl); g = 1/(1+u)            # sigmoid without Sigmoid LUT
    out      = g * (x + (E * u * r) @ Wv)       # since (1-g)/g == u exactly

Numerics: x is split on the host into xh (fp16) + xl (residual). Scores are
computed at fp32-grade precision as three PE passes, all scaled by 2^12 so
the corrections fit fp8 range (the scale is unwound in the exp/gate stats):
  (2^12*Gh)@xh [fp16, full rate] + fp8(2^12*Gl)@fp8(xh) + fp8(Gh)@fp8(2^12*xl)
with both correction passes in fp8 DoubleRow mode (2 chunks per matmul at 2x
rate). fp8(xh) is cast on-device on the Scalar engine to save HBM traffic.

Combine: the PE accumulates xh (identity matmul) and the retrieval (fp16,
K padded to 128 with zero rows against a once-zeroed E buffer) into the same
PSUM bank, so the elementwise work collapses to one DVE multiply by a
broadcast gate row per bank pair; output is written fp16 (host upcasts).

Schedule per iteration t (software pipelined, all DMAs fully contiguous):
  PE : id-quad0(t) -> ET(t) -> g_bc(t) -> [P quads(t) + S(t+1) interleaved,
       same-bank matmuls >= 6 apart] -> Stok(t+1) transposes
  DVE: combine muls(t) -> stats(t+1)  (emitted after quad-2 TTs)
  ACT: E_copy(t), g2(t), S_copy(t+1), exps(t+1), fp8-cast(t+2)
  GpS: output DMA issue (SWDGE)      Sync: input DMA issue (HWDGE)
Each core handles one batch element; x is pre-tiled on the host so every
transfer is a single 1-2 MiB contiguous DMA.
"""

from contextlib import ExitStack

import numpy as np

F16_NP = np.float16
import ml_dtypes

F8_NP = ml_dtypes.float8_e4m3

import concourse.bass as bass
import concourse.tile as tile
from concourse import bacc
from concourse import mybir
from concourse.bass import ts
from concourse.bass_utils import run_bass_kernel_spmd
from concourse.masks import make_identity

F32 = mybir.dt.float32
F32R = mybir.dt.float32r
F16 = mybir.dt.float16
F8 = mybir.dt.float8e4
DR = mybir.MatmulPerfMode.DoubleRow
SIG = 4096.0  # power-of-2 score scale so the fp8 correction passes fit fp8 range
AX_X = mybir.AxisListType
ALU = mybir.AluOpType
ACTF = mybir.ActivationFunctionType

B = 8
L = 4096
DIM = 2048
NSLOT = 64
NCH = DIM // 128  # 16 dim chunks
TOK = 512  # tokens per tile
NT = L // TOK  # 8 tiles per core
NQ = TOK // 128  # 4 token quarters per tile


def _build(gate_b: float, interleave: bool = True) -> bass.Bass:
    nc = bacc.Bacc("TRN2", target_bir_lowering=False, debug=False)

    xhd = nc.dram_tensor("xhd", [NT * 128, NCH * TOK], F16, kind="ExternalInput").ap()
    xl8d = nc.dram_tensor(
        "xl8d", [NT * 128, NCH * TOK], F8, kind="ExternalInput"
    ).ap()
    Ghd = nc.dram_tensor(
        "Ghd", [128, NCH * 128], F16, kind="ExternalInput"
    ).ap()
    Gh8d = nc.dram_tensor("Gh8d", [128, NCH * 128], F8, kind="ExternalInput").ap()
    Gl8d = nc.dram_tensor("Gl8d", [128, NCH * 128], F8, kind="ExternalInput").ap()
    Wvd = nc.dram_tensor("Wvd", [128, NCH * 128], F16, kind="ExternalInput").ap()
    gvd = nc.dram_tensor("gvd", [1, NQ * NSLOT], F32, kind="ExternalInput").ap()
    outb = nc.dram_tensor(
        "outb", [NT * 128, NCH * TOK], F16, kind="ExternalOutput"
    ).ap()

    # per-tile views; dim d = c*128 + p everywhere
    xh_v = xhd.rearrange("(t p) (c k) -> t p c k", p=128, k=TOK)
    xl8_v = xl8d.rearrange("(t p) (c k) -> t p c k", p=128, k=TOK)
    outb_v = outb.rearrange("(t p) (c k) -> t p c k", p=128, k=TOK)
    Gh_v = Ghd.rearrange("p (c n) -> p c n", n=128)
    Gh8_v = Gh8d.rearrange("p (c n) -> p c n", n=128)
    Gl8_v = Gl8d.rearrange("p (c n) -> p c n", n=128)
    Wv_v = Wvd.rearrange("n (c j) -> n c j", j=128)

    with tile.TileContext(nc) as tc, ExitStack() as ctx:
        consts = ctx.enter_context(tc.tile_pool(name="consts", bufs=1))
        xpool = ctx.enter_context(tc.tile_pool(name="xpool", bufs=4))
        x8pool = ctx.enter_context(tc.tile_pool(name="x8pool", bufs=3))
        opool = ctx.enter_context(tc.tile_pool(name="opool", bufs=3))
        work = ctx.enter_context(tc.tile_pool(name="work", bufs=2))
        small = ctx.enter_context(tc.tile_pool(name="small", bufs=2))
        psS = ctx.enter_context(tc.tile_pool(name="psS", bufs=1, space="PSUM"))
        psT = ctx.enter_context(tc.tile_pool(name="psT", bufs=1, space="PSUM"))
        psE = ctx.enter_context(tc.tile_pool(name="psE", bufs=1, space="PSUM"))
        psG = ctx.enter_context(tc.tile_pool(name="psG", bufs=1, space="PSUM"))
        psP = ctx.enter_context(tc.tile_pool(name="psP", bufs=2, space="PSUM"))

        ident = consts.tile([128, 128], F32)
        make_identity(nc, ident)
        ident_16 = consts.tile([128, 128], F16)
        nc.vector.tensor_copy(ident_16, ident)
        Gh_sb = consts.tile([128, NCH, 128], F16)
        Gh8_sb = consts.tile([128, NCH, 128], F8)
        Gl8_sb = consts.tile([128, NCH, 128], F8)
        Wv_sb = consts.tile([128, NCH, 128], F16)
        # manually double-buffered full-height E tiles; rows 65:128 are
        # zeroed once and never rewritten, so the zero-padded Wv rows
        # always multiply zeros (K=128 keeps the weight load on the
        # fast path)
        E_pair = []
        for i in range(2):
            E_buf = consts.tile([128, NQ, 128], F16, name=f"E_buf{i}")
            nc.vector.memset(E_buf, 0.0)
            E_pair.append(E_buf)
        gv4_flat = consts.tile([128, NQ * NSLOT], F32)
        gv4 = gv4_flat.rearrange("p (a b) -> p a b", b=NSLOT)
        ones_16 = consts.tile([NSLOT + 1, 128], F16)
        nc.vector.memset(ones_16, 1.0)

        def ph_dma(t):
            xh_sb = xpool.tile([128, NCH, TOK], F16, tag="xh")
            nc.sync.dma_start(out=xh_sb, in_=xh_v[t])
            xl8_sb = x8pool.tile([128, NCH, TOK], F8, tag="xl8")
            nc.sync.dma_start(out=xl8_sb, in_=xl8_v[t])
            return {"xh": xh_sb, "xl8": xl8_sb}

        def ph_cast(t, st):
            # fp8 copy of xh on ACT (saves 1 MiB/tile of HBM traffic)
            xh8_sb = x8pool.tile([128, NCH, TOK], F8, tag="xh8")
            nc.scalar.copy(xh8_sb, st["xh"])
            st["xh8"] = xh8_sb

        def s_seq(st):
            # sigma-scaled split: SIG*G@x = (SIG*Gh)@xh [f16]
            #   + fp8(SIG*Gl)@fp8(xh) + fp8(Gh)@fp8(SIG*xl)  [fp8 DoubleRow,
            #   2 chunks per matmul at 2x rate]
            seq = []
            for c in range(NCH):
                seq.append((Gh_sb[:, c, :], st["xh"][:, c, :], None))
            for G8, xkey in ((Gl8_sb, "xh8"), (Gh8_sb, "xl8")):
                for c2 in range(NCH // 2):
                    seq.append(
                        (
                            G8[:, 2 * c2 : 2 * c2 + 2, :],
                            st[xkey][:, 2 * c2 : 2 * c2 + 2, :],
                            DR,
                        )
                    )
            return seq

        def ph_S(t, st):
            """S^T = [Ws; gxw] @ x_tile via 3 fp16 passes per chunk."""
            S_ps = psS.tile([128, TOK], F32, tag="S")
            seq = s_seq(st)
            for i, (lhsT, rhs, pm) in enumerate(seq):
                nc.tensor.matmul(
                    S_ps,
                    lhsT,
                    rhs,
                    start=(i == 0),
                    stop=(i == len(seq) - 1),
                    perf_mode=pm,
                )
            st["S_ps"] = S_ps

        def ph_Scopy(t, st):
            S_ps = st.pop("S_ps")
            S_sb = work.tile([NSLOT + 1, TOK], F32, tag="S_sb")
            nc.scalar.copy(S_sb, S_ps[0 : NSLOT + 1, :])
            st["S_sb"] = S_sb

        def ph_Stok_T(t, st):
            S_sb = st.pop("S_sb")
            Stok = psT.tile([128, NQ, NSLOT + 1], F32, tag="T")
            for q in range(NQ):
                nc.tensor.transpose(
                    Stok[:, q, :],
                    S_sb[:, ts(q, 128)],
                    ident[0 : NSLOT + 1, 0 : NSLOT + 1],
                )
            st["Stok"] = Stok

        def ph_Stok(t, st):
            ph_Scopy(t, st)
            ph_Stok_T(t, st)

        def ph_stats(t, st):
            """Token-major softmax stats; produces Ec (bf16, col 64 = g)."""
            Stok = st.pop("Stok")
            mx4 = small.tile([128, NQ], F32, tag="mx4")
            nc.vector.tensor_reduce(mx4, Stok[:, :, 0:NSLOT], axis=AX_X.X, op=ALU.max)
            mb4 = small.tile([128, NQ], F32, tag="mb4")
            nc.vector.tensor_scalar_mul(mb4, mx4, -10.0 / SIG)
            Etok = work.tile([128, NQ, NSLOT], F32, tag="Etok")
            s4 = small.tile([128, NQ], F32, tag="s4")
            for q in range(NQ):
                nc.scalar.activation(
                    Etok[:, q, :],
                    Stok[:, q, 0:NSLOT],
                    func=ACTF.Exp,
                    bias=mb4[:, q : q + 1],
                    scale=10.0 / SIG,
                    accum_out=s4[:, q : q + 1],
                )
            gvd4 = small.tile([128, NQ], F32, tag="gvd4")
            scr = work.tile([128, NQ, NSLOT], F32, tag="scr")
            nc.vector.tensor_mul(scr, Etok, gv4)
            nc.vector.tensor_reduce(gvd4, scr, axis=AX_X.X, op=ALU.add)
            r4 = small.tile([128, NQ], F32, tag="r4")
            nc.vector.reciprocal(r4, s4)
            t4 = small.tile([128, NQ], F32, tag="t4")
            nc.vector.tensor_mul(t4, gvd4, r4)
            gxs = small.tile([128, NQ], F32, tag="gxs")
            gx_row = Stok[:, :, NSLOT : NSLOT + 1].rearrange("p a b -> p (a b)")
            nc.vector.tensor_scalar_mul(gxs, gx_row, 1.0 / SIG)
            gl4 = small.tile([128, NQ], F32, tag="gl4")
            nc.vector.tensor_add(gl4, t4, gxs)
            u4 = small.tile([128, NQ], F32, tag="u4")
            nc.scalar.activation(
                u4, gl4, func=ACTF.Exp, bias=-gate_b, scale=-1.0
            )
            den4 = small.tile([128, NQ], F32, tag="den4")
            nc.vector.tensor_scalar_add(den4, u4, 1.0)
            g4 = small.tile([128, NQ], F32, tag="g4")
            nc.vector.reciprocal(g4, den4)
            cp4 = small.tile([128, NQ], F32, tag="cp4")
            nc.vector.tensor_mul(cp4, u4, r4)
            Ec = work.tile([128, NQ, NSLOT + 1], F16, tag="Ec")
            for q in range(NQ):
                nc.vector.tensor_scalar_mul(
                    Ec[:, q, 0:NSLOT], Etok[:, q, :], cp4[:, q : q + 1]
                )
            g_col = Ec[:, :, NSLOT : NSLOT + 1].rearrange("p a b -> p (a b)")
            nc.vector.tensor_copy(g_col, g4)
            st["Ec"] = Ec

        def ph_combine(t, st, s_next=None, stats_cb=None):
            """P = x + E''@Wv on PE in quads (2 PSUM pairs in flight ->
            same-bank matmuls >=6 apart); out = g * P on DVE. The
            iteration's PE stream opens with dependency-free ident
            matmuls while DVE finishes the previous stats; the next
            tile's S matmuls fill quads 0-2 and its stats are emitted
            mid-iteration."""
            x = st["xh"]
            Ec = st.pop("Ec")
            E_sb = E_pair[t % 2]
            E_flat = E_sb.rearrange("p a b -> p (a b)")  # [128, 512]
            if s_next is not None:
                t2, st2 = s_next
                st2["_Sps"] = psS.tile([128, TOK], F32, tag="S", name="S_ps_il")
                st2["_seq"] = s_seq(st2)

            def s_il(n):
                if s_next is None:
                    return
                seq = st2["_seq"]
                lo = st2.setdefault("_si", 0)
                hi = min(lo + n, len(seq))
                st2["_si"] = hi
                for i2 in range(lo, hi):
                    lhsT, rhs, pm = seq[i2]
                    nc.tensor.matmul(
                        st2["_Sps"],
                        lhsT,
                        rhs,
                        start=(i2 == 0),
                        stop=(i2 == len(seq) - 1),
                        perf_mode=pm,
                        skip_group_check=True,
                    )
                if lo < len(seq) <= hi:
                    st2["S_ps"] = st2.pop("_Sps")
                    ph_Scopy(t2, st2)

            out_sb = opool.tile([128, NCH, TOK], F16, tag="o")
            quads = []
            for qd in range(4):
                Pa = psP.tile([128, 2, TOK], F32, tag="P", name="Pa")
                Pb = psP.tile([128, 2, TOK], F32, tag="P", name="Pb")
                quads.append((Pa, Pb))
                c0 = 4 * qd
                for k in range(2):
                    nc.tensor.matmul(
                        Pa[:, k, :], ident_16, x[:, c0 + k, :],
                        start=True, stop=False,
                    )
                for k in range(2):
                    nc.tensor.matmul(
                        Pb[:, k, :], ident_16, x[:, c0 + 2 + k, :],
                        start=True, stop=False,
                    )
                if qd == 0:
                    # ET transposes + g broadcast ride here: the idents
                    # above gave DVE time to finish this tile's stats
                    ET = psE.tile([128, NQ, 128], F16, tag="ET")
                    for q in range(NQ):
                        nc.tensor.transpose(
                            ET[0 : NSLOT + 1, q, :], Ec[:, q, :], ident_16
                        )
                    nc.scalar.copy(
                        E_sb[0 : NSLOT + 1, :, :], ET[0 : NSLOT + 1, :, :]
                    )
                    g_ps = psG.tile([128, TOK], F32, tag="G")
                    nc.tensor.matmul(
                        g_ps,
                        ones_16[NSLOT : NSLOT + 1, :],
                        E_flat[NSLOT : NSLOT + 1, :],
                        start=True,
                        stop=True,
                    )
                    g2 = work.tile([128, 2, TOK], F32, tag="g2")
                    nc.scalar.copy(g2[:, 0, :], g_ps)
                    nc.scalar.copy(g2[:, 1, :], g_ps)
                else:
                    s_il(11)
                for k in range(2):
                    nc.tensor.matmul(
                        Pa[:, k, :], Wv_sb[:, c0 + k, :], E_flat,
                        start=False, stop=True,
                    )
                for k in range(2):
                    nc.tensor.matmul(
                        Pb[:, k, :], Wv_sb[:, c0 + 2 + k, :], E_flat,
                        start=False, stop=True,
                    )
                s_il(10 if qd == 0 else 11)
                if qd == 2 and s_next is not None:
                    ph_Stok_T(t2, st2)
                nc.vector.tensor_mul(out_sb[:, c0 : c0 + 2, :], Pa, g2)
                nc.vector.tensor_mul(out_sb[:, c0 + 2 : c0 + 4, :], Pb, g2)
                if qd == 0 and s_next is None:
                    nc.gpsimd.dma_start(
                        out=outb_v[t][:, 0:4, :], in_=out_sb[:, 0:4, :]
                    )
                elif qd == 1:
                    if s_next is None:
                        nc.gpsimd.dma_start(
                            out=outb_v[t][:, 4:8, :], in_=out_sb[:, 4:8, :]
                        )
                    else:
                        nc.gpsimd.dma_start(
                            out=outb_v[t][:, 0:8, :], in_=out_sb[:, 0:8, :]
                        )
                elif qd == 2:
                    if stats_cb is not None:
                        stats_cb()
                    if s_next is None:
                        nc.gpsimd.dma_start(
                            out=outb_v[t][:, 8:12, :], in_=out_sb[:, 8:12, :]
                        )
            if s_next is None:
                nc.gpsimd.dma_start(
                    out=outb_v[t][:, 12:16, :], in_=out_sb[:, 12:16, :]
                )
            else:
                nc.gpsimd.dma_start(
                    out=outb_v[t][:, 8:16, :], in_=out_sb[:, 8:16, :]
                )
            if s_next is not None:
                st2.pop("_seq")
                st2.pop("_si", None)

        # Software pipeline: per iteration t the PE stream is
        #   ET(t) -> g_bc(t) -> [P pairs(t) + S(t+1) interleaved] -> Stok(t+1)
        # DVE: combine TTs(t) -> stats(t+1);  x DMA runs 2 tiles ahead.
        # Prologue DMAs are issued in latency-critical order so S(0) can
        # start after just xh(0) + Gh (~2.5 MiB).
        st0 = {}
        st0["xh"] = xpool.tile([128, NCH, TOK], F16, tag="xh", name="xh0")
        nc.sync.dma_start(out=st0["xh"], in_=xh_v[0])
        nc.sync.dma_start(out=Gh_sb, in_=Gh_v)
        st0["xl8"] = x8pool.tile([128, NCH, TOK], F8, tag="xl8", name="xl80")
        nc.sync.dma_start(out=st0["xl8"], in_=xl8_v[0])
        nc.sync.dma_start(out=Gh8_sb, in_=Gh8_v)
        nc.sync.dma_start(out=Gl8_sb, in_=Gl8_v)
        nc.sync.dma_start(out=Wv_sb, in_=Wv_v)
        nc.sync.dma_start(out=gv4_flat, in_=gvd.to_broadcast((128, NQ * NSLOT)))
        states = {0: st0, 1: ph_dma(1), 2: ph_dma(2)}
        ph_cast(0, states[0])
        ph_cast(1, states[1])
        ph_S(0, states[0])
        ph_cast(2, states[2])
        ph_Stok(0, states[0])
        ph_stats(0, states[0])
        for t in range(NT):
            if t + 3 < NT:
                states[t + 3] = ph_dma(t + 3)
            cb = None
            if t + 1 < NT and interleave:
                cb = lambda t1=t + 1: ph_stats(t1, states[t1])
            ph_combine(
                t,
                states[t],
                s_next=(t + 1, states[t + 1])
                if (interleave and t + 1 < NT)
                else None,
                stats_cb=cb,
            )
            if 0 < t and t + 2 < NT:
                ph_cast(t + 2, states[t + 2])
            if t + 1 < NT and not interleave:
                ph_S(t + 1, states[t + 1])
                ph_Stok(t + 1, states[t + 1])
                ph_stats(t + 1, states[t + 1])
            del states[t]

    nc.compile()
    return nc


def _fold_weights(memory, key_w, value_w, gate_w):
    mem = np.asarray(memory, np.float64)
    # query = x @ key_w.T ; scores = query @ memory.T = x @ (memory @ key_w).T
    Ws = (mem @ np.asarray(key_w, np.float64)).astype(np.float32)  # [64, 2048]
    Wv = (mem @ np.asarray(value_w, np.float64).T).astype(np.float32)  # [64, 2048]
    gx = np.asarray(gate_w[0, :DIM], dtype=np.float32)
    gv = (Wv.astype(np.float64) @ np.asarray(gate_w[0, DIM:], np.float64)).astype(
        np.float32
    )
    G = np.concatenate(
        [Ws, gx[None, :], np.zeros((128 - NSLOT - 1, DIM), np.float32)], axis=0
    )  # [128, 2048]: 64 slots, gate row, zero padding (FWL wants 128 cols)
    GT = np.ascontiguousarray(
        G.T.reshape(NCH, 128, 128).transpose(1, 0, 2)
    ).reshape(128, NCH * 128)
    Gh = GT.astype(np.float16)
    GhF = Gh.astype(np.float32)
    Ghs = (GhF * 4096.0).astype(np.float16)  # exact power-of-2 shift
    Gh8 = GhF.astype(F8_NP)
    Gl8 = ((GT - GhF) * 4096.0).astype(F8_NP)
    WvT = np.ascontiguousarray(Wv.reshape(NSLOT, NCH * 128))  # [64, 2048]
    gv4 = np.tile(gv, NQ).reshape(1, NQ * NSLOT)
    return Ghs, Gh8, Gl8, WvT, gv4


def _tile_x(xb):
    # [L, D] -> [NT*128, NCH*TOK]: blob[t, p, c, k] = x[t*TOK+k, c*128+p]
    return np.ascontiguousarray(
        xb.reshape(NT, TOK, NCH, 128).transpose(0, 3, 2, 1)
    ).reshape(NT * 128, NCH * TOK)


def _untile_out(blob):
    # [NT*128, NCH*TOK] -> [L, D]
    return (
        blob.reshape(NT, 128, NCH, TOK)
        .transpose(0, 3, 2, 1)
        .reshape(L, DIM)
        .astype(np.float32)
    )


def kernel(
    x,
    memory,
    key_w,
    value_w,
    gate_w,
    gate_b,
    _trace=False,
    _tmpdir=None,
):
    x = np.asarray(x, dtype=np.float32)
    Ghs, Gh8, Gl8, WvT, gv4 = _fold_weights(
        np.asarray(memory, np.float32),
        np.asarray(key_w, np.float32),
        np.asarray(value_w, np.float32),
        np.asarray(gate_w, np.float32),
    )
    Wv_16 = np.concatenate([WvT, np.zeros_like(WvT)], axis=0).astype(F16_NP)
    nc = _build(float(np.asarray(gate_b).reshape(-1)[0]))
    in_maps = []
    for b in range(B):
        xt = _tile_x(x[b])
        xh = xt.astype(np.float16)
        xl8 = ((xt - xh.astype(np.float32)) * 4096.0).astype(F8_NP)
        in_maps.append(
            {
                "xhd": xh,
                "xl8d": xl8,
                "Ghd": Ghs,
                "Gh8d": Gh8,
                "Gl8d": Gl8,
                "Wvd": Wv_16,
                "gvd": gv4,
            }
        )
    res = run_bass_kernel_spmd(
        nc, in_maps, list(range(B)), trace=_trace, tmpdir=_tmpdir
    )
    out = np.stack(
        [_untile_out(np.asarray(res.results[b]["outb"])) for b in range(B)], axis=0
    )
    if _trace:
        return out.astype(np.float32), res
    return out.astype(np.float32)
